# revision 1
# baseline (speedup 1.0000x reference)
"""Causal self-attention with sink, sharded over 8 TRN2 NeuronCores.

Sharding: batch x head-group. Core c handles batch b=c//4 and heads
[4*(c%4), 4*(c%4)+4). Each core computes its QKV projection slice,
attention for its 4 heads, and a partial output projection; the host sums
the 4 partials per batch.

Device layout (per core), everything "transposed" (T on the free dim):
  - xT   [C=1024, T=2048]   (host pre-transposed x[b])
  - qT/kT in SBUF as head-pair tiles [128, T] (2 heads x 64 stacked)
  - v1   [128, 16, 4, 65]   v in natural [t, d] layout per tk-chunk/head
                            plus a ones column (65th) that accumulates the
                            softmax denominator inside the PV matmul
  - S^T = K^T Q per (head, tq-block, tk-chunk) -> exp -> E^T (no max
    subtraction: logits are O(1) for this problem's scale)
  - PV:  out^T[d, tq] (+ denom row) accumulated in PSUM over tk-chunks;
    sink term enters the denominator via a rank-1 matmul
  - normalize via reciprocal + gpsimd partition_broadcast + multiply
  - out projection produces natural [t, co] partials via yT-as-stationary
Matmuls run in float32r (fast 4-byte PE streaming mode).
"""

import os
import sys

import numpy as np

B, T, C = 2, 2048, 1024
H, D = 16, 64
NCORES = 8
HLOC = 4           # heads per core
GQ = HLOC * D      # 256 per-core q (or k or v) features
F = 3 * GQ         # 768 per-core qkv features
NCC = C // 128     # 8 contraction chunks
NTQ = T // 512     # 4 query blocks
NTK = T // 128     # 16 key chunks
SCALE = 1.0 / np.sqrt(D)

_BASS_PATHS = ("/opt/trn_rl_repo", "/root/.axon_site/_ro/trn_rl_repo")


def _import_bass():
    for p in _BASS_PATHS:
        if os.path.isdir(p) and p not in sys.path:
            sys.path.insert(0, p)
    import concourse.bass as bass
    import concourse.mybir as mybir
    import concourse.tile as tile
    from concourse import bacc
    return bass, mybir, tile, bacc


def build_nc(mm_dt="float32r", with_bias_qkv=True, with_bias_proj=True):
    """Build the per-core Bass program (same program for all 8 cores)."""
    bass, mybir, tile, bacc = _import_bass()
    f32 = mybir.dt.float32
    mdt = getattr(mybir.dt, mm_dt)
    AF = mybir.ActivationFunctionType

    nc = bacc.Bacc("TRN2", target_bir_lowering=False, debug=False)

    xT = nc.dram_tensor("xT", [C, T], mdt, kind="ExternalInput")
    wqkvT = nc.dram_tensor("wqkvT", [C, F], mdt, kind="ExternalInput")
    bqkv = nc.dram_tensor("bqkv", [1, F], mdt, kind="ExternalInput")
    wpT = nc.dram_tensor("wpT", [GQ, C], mdt, kind="ExternalInput")
    bp = nc.dram_tensor("bp", [1, C], mdt, kind="ExternalInput")
    sinkrow = nc.dram_tensor("sinkrow", [1, HLOC * 512], mdt, kind="ExternalInput")
    e65 = nc.dram_tensor("e65", [1, 65], mdt, kind="ExternalInput")
    out = nc.dram_tensor("out", [T, C], f32, kind="ExternalOutput")

    with tile.TileContext(nc) as tc:
        with (
            tc.tile_pool(name="const", bufs=1) as const,
            tc.tile_pool(name="persist", bufs=1) as persist,
        ):
            # ---- constants ----------------------------------------------
            # (memset does not support float32r, so memset fp32 scratch and
            #  engine-copy into the f32r tiles)
            ones_f = const.tile([1, 512], f32, tag="ones_f")
            nc.vector.memset(ones_f, 1.0)
            ones_r = const.tile([1, 512], mdt, tag="ones")
            nc.vector.tensor_copy(out=ones_r[:], in_=ones_f[:])
            ones_col = const.tile([128, 1], f32, tag="ones_col")
            nc.vector.memset(ones_col, 1.0)
            e65_r = const.tile([1, 65], mdt, tag="e65")
            nc.sync.dma_start(out=e65_r[:], in_=e65[:, :])
            bqkv_r = const.tile([1, F], mdt, tag="bqkv")
            nc.sync.dma_start(out=bqkv_r[:], in_=bqkv[:, :])
            bp_r = const.tile([1, C], mdt, tag="bp")
            nc.sync.dma_start(out=bp_r[:], in_=bp[:, :])
            sink_r = const.tile([1, HLOC * 512], mdt, tag="sink")
            nc.sync.dma_start(out=sink_r[:], in_=sinkrow[:, :])
            # additive causal mask for the 128x128 diagonal blocks of S^T
            # (tk on partitions, tq on free): keep where tq >= tk.
            tri = const.tile([128, 128], f32, tag="tri")
            nc.gpsimd.memset(tri, 0.0)
            nc.gpsimd.affine_select(
                out=tri, in_=tri,
                compare_op=mybir.AluOpType.is_ge,
                fill=-1e30,
                base=0,
                pattern=[[1, 128]],
                channel_multiplier=-1,
            )

            # ---- persistent activations ---------------------------------
            # qk feature-block tiles: [q01, q23, k01, k23] each [128, T]
            qk = [persist.tile([128, T], mdt, tag=f"qk{i}", name=f"qk{i}") for i in range(4)]
            # v natural + ones column
            v1 = persist.tile([128, NTK, HLOC, 65], mdt, tag="v1", name="v1")
            nc.vector.tensor_copy(
                out=v1[:, :, :, 64:65],
                in_=ones_col[:, :].to_broadcast([128, NTK, HLOC, 1]),
            )
            # normalized attention output, head pairs stacked: yT[hp] [128, T]
            yT = [persist.tile([128, T], mdt, tag=f"yT{i}", name=f"yT{i}") for i in range(2)]

            with (
                tc.tile_pool(name="xpool", bufs=1) as xpool,
                tc.tile_pool(name="wqkvpool", bufs=1) as wqp,
            ):
                wq = []
                for i in range(NCC):
                    t = wqp.tile([128, F], mdt, tag=f"wqkv{i}", name=f"wqkv{i}")
                    nc.sync.dma_start(out=t[:], in_=wqkvT[128 * i:128 * (i + 1), :])
                    wq.append(t)
                xt = []
                for i in range(NCC):
                    t = xpool.tile([128, T], mdt, tag=f"xt{i}", name=f"xt{i}")
                    nc.sync.dma_start(out=t[:], in_=xT[128 * i:128 * (i + 1), :])
                    xt.append(t)

                # ---- q/k projection (transposed out: [feat, t]) ---------
                # feature blocks in wqkvT cols: q:[0,256) k:[256,512) v:[512,768)
                # qk[0]=q01 cols [0,128), qk[1]=q23 [128,256), qk[2]=k01 [256,384), qk[3]=k23 [384,512)
                with tc.tile_pool(name="projps", bufs=4, space="PSUM") as pp:
                    fb_order = [0, 2, 1, 3]  # q01, k01, q23, k23
                    for fb in fb_order:
                        col0 = [0, 128, 256, 384][fb]
                        dst = [0, 1, 2, 3][fb]
                        for tqi in range(NTQ):
                            ps = pp.tile([128, 512], f32, tag="ps")
                            for cc in range(NCC):
                                nc.tensor.matmul(
                                    ps[:, :],
                                    wq[cc][:, col0:col0 + 128],
                                    xt[cc][:, 512 * tqi:512 * (tqi + 1)],
                                    start=(cc == 0),
                                    stop=(cc == NCC - 1 and not with_bias_qkv),
                                )
                            if with_bias_qkv:
                                nc.tensor.matmul(
                                    ps[:, :],
                                    bqkv_r[:, col0:col0 + 128],
                                    ones_r[:, 0:512],
                                    start=False, stop=True,
                                )
                            nc.any.tensor_copy(
                                out=qk[dst][:, 512 * tqi:512 * (tqi + 1)], in_=ps[:, :]
                            )

                    # ---- v projection (natural out: [t, feat]) ----------
                    for tb in range(NTK):
                        ps = pp.tile([128, GQ], f32, tag="psv")
                        for cc in range(NCC):
                            nc.tensor.matmul(
                                ps[:, :],
                                xt[cc][:, 128 * tb:128 * (tb + 1)],
                                wq[cc][:, 512:768],
                                start=(cc == 0),
                                stop=(cc == NCC - 1 and not with_bias_qkv),
                            )
                        if with_bias_qkv:
                            nc.tensor.matmul(
                                ps[:, :],
                                ones_r[:, 0:128],
                                bqkv_r[:, 512:768],
                                start=False, stop=True,
                            )
                        nc.any.tensor_copy(
                            out=v1[:, tb, :, 0:64],
                            in_=ps[:].rearrange("p (h d) -> p h d", h=HLOC),
                        )

            # ---- attention ----------------------------------------------
            with (
                tc.tile_pool(name="spool", bufs=4, space="PSUM") as sp,
                tc.tile_pool(name="pvpool", bufs=2, space="PSUM") as pvp,
                tc.tile_pool(name="epool", bufs=6) as ep,
                tc.tile_pool(name="rpool", bufs=4) as rp,
            ):
                for h in range(HLOC):
                    hp, hs = divmod(h, 2)
                    pb = 64 * hs  # partition base of this head inside the pair tiles
                    q_t, k_t = qk[hp], qk[2 + hp]
                    for tqi in range(NTQ):
                        tq0 = 512 * tqi
                        pv = pvp.tile([65, 512], f32, tag="pv")
                        # sink term -> denominator row (also zero-fills rows 0..63)
                        nc.tensor.matmul(
                            pv[:, :], e65_r[:, :], sink_r[0:1, h * 512:(h + 1) * 512],
                            start=True, stop=False,
                        )
                        for tki in range(tq0 // 128 + 4):
                            tk0 = 128 * tki
                            last = tki == tq0 // 128 + 3
                            if tk0 < tq0:
                                # fully-visible block
                                s = sp.tile([128, 512], f32, tag="s")
                                nc.tensor.matmul(
                                    s[:, :],
                                    k_t[pb:pb + 64, tk0:tk0 + 128],
                                    q_t[pb:pb + 64, tq0:tq0 + 512],
                                    start=True, stop=True,
                                )
                                e = ep.tile([128, 512], mdt, tag="e")
                                nc.scalar.activation(out=e[:, :], in_=s[:, :],
                                                     func=AF.Exp, scale=SCALE)
                                nc.tensor.matmul(
                                    pv[:, :], v1[:, tki, h, :], e[:, :],
                                    start=False, stop=last,
                                )
                            else:
                                # diagonal-region block: visible tq cols [tk0, tq0+512)
                                m = (tk0 - tq0) // 128
                                w = 512 - 128 * m
                                s = sp.tile([128, 512], f32, tag="s")
                                nc.tensor.matmul(
                                    s[:, 0:w],
                                    k_t[pb:pb + 64, tk0:tk0 + 128],
                                    q_t[pb:pb + 64, tq0 + 128 * m:tq0 + 512],
                                    start=True, stop=True,
                                )
                                # causal mask on the first 128 visible cols
                                nc.vector.tensor_add(out=s[:, 0:128], in0=s[:, 0:128], in1=tri)
                                e = ep.tile([128, 512], mdt, tag="e")
                                nc.scalar.activation(out=e[:, 0:w], in_=s[:, 0:w],
                                                     func=AF.Exp, scale=SCALE)
                                nc.tensor.matmul(
                                    pv[:, 128 * m:512], v1[:, tki, h, :], e[:, 0:w],
                                    start=False, stop=last,
                                )
                        # normalize: y = out / denom
                        r1 = rp.tile([1, 512], f32, tag="r1")
                        nc.vector.reciprocal(out=r1, in_=pv[64:65, :])
                        rb = rp.tile([64, 512], f32, tag="rb")
                        nc.gpsimd.partition_broadcast(rb, r1)
                        nc.vector.tensor_mul(
                            out=yT[hp][pb:pb + 64, tq0:tq0 + 512],
                            in0=pv[0:64, :],
                            in1=rb,
                        )

            # ---- output projection (natural [t, co]) --------------------
            with (
                tc.tile_pool(name="opps", bufs=4, space="PSUM") as op,
                tc.tile_pool(name="ostage", bufs=3) as ost,
                tc.tile_pool(name="wppool", bufs=1) as wpp,
            ):
                wp = []
                for i in range(GQ // 128):
                    t = wpp.tile([128, C], mdt, tag=f"wp{i}", name=f"wp{i}")
                    nc.sync.dma_start(out=t[:], in_=wpT[128 * i:128 * (i + 1), :])
                    wp.append(t)
                for tb in range(NTK):
                    stg = ost.tile([128, C], f32, tag="ostg")
                    for co in range(2):
                        ps = op.tile([128, 512], f32, tag="ops")
                        for hd in range(2):
                            nc.tensor.matmul(
                                ps[:, :],
                                yT[hd][:, 128 * tb:128 * (tb + 1)],
                                wp[hd][:, 512 * co:512 * (co + 1)],
                                start=(hd == 0),
                                stop=(hd == 1 and not with_bias_proj),
                            )
                        if with_bias_proj:
                            nc.tensor.matmul(
                                ps[:, :],
                                ones_r[:, 0:128],
                                bp_r[:, 512 * co:512 * (co + 1)],
                                start=False, stop=True,
                            )
                        nc.any.tensor_copy(out=stg[:, 512 * co:512 * (co + 1)], in_=ps[:, :])
                    nc.sync.dma_start(out=out[128 * tb:128 * (tb + 1), :], in_=stg[:, :])

    nc.finalize()
    return nc


def make_core_inputs(x, W_qkv, b_qkv, W_proj, b_proj, sink_logit):
    """Host-side sharding: per-core input dicts (host does the transposes)."""
    x = np.asarray(x, dtype=np.float32)
    W_qkv = np.asarray(W_qkv, dtype=np.float32)
    b_qkv = np.asarray(b_qkv, dtype=np.float32)
    W_proj = np.asarray(W_proj, dtype=np.float32)
    b_proj = np.asarray(b_proj, dtype=np.float32)
    sink_logit = np.asarray(sink_logit, dtype=np.float32)

    xTs = [np.ascontiguousarray(x[b].T) for b in range(B)]
    e65 = np.zeros((1, 65), dtype=np.float32)
    e65[0, 64] = 1.0

    in_maps = []
    for c in range(NCORES):
        b, g = divmod(c, 4)
        h0 = HLOC * g
        q_rows = slice(GQ * g, GQ * (g + 1))
        k_rows = slice(C + GQ * g, C + GQ * (g + 1))
        v_rows = slice(2 * C + GQ * g, 2 * C + GQ * (g + 1))
        w_slice = np.concatenate(
            [W_qkv[q_rows], W_qkv[k_rows], W_qkv[v_rows]], axis=0
        )  # (768, 1024)
        b_slice = np.concatenate(
            [b_qkv[q_rows], b_qkv[k_rows], b_qkv[v_rows]], axis=0
        )  # (768,)
        sink = np.repeat(
            np.exp(sink_logit[h0:h0 + HLOC]).astype(np.float32)[:, None], 512, axis=1
        ).reshape(1, HLOC * 512)
        in_maps.append({
            "xT": xTs[b],
            "wqkvT": np.ascontiguousarray(w_slice.T),
            "bqkv": b_slice[None, :].copy(),
            "wpT": np.ascontiguousarray(W_proj[:, q_rows].T),
            "bp": (b_proj if g == 0 else np.zeros_like(b_proj))[None, :].copy(),
            "sinkrow": sink,
            "e65": e65,
        })
    return in_maps


_NC_CACHE = {}


def kernel(x, W_qkv, b_qkv, W_proj, b_proj, sink_logit, _trace=False):
    from concourse.bass_utils import run_bass_kernel_spmd  # noqa: F401 (path set below)

    in_maps = make_core_inputs(x, W_qkv, b_qkv, W_proj, b_proj, sink_logit)
    with_bias_qkv = bool(np.any(np.asarray(b_qkv)))
    with_bias_proj = bool(np.any(np.asarray(b_proj)))
    key = ("float32r", with_bias_qkv, with_bias_proj)
    if key not in _NC_CACHE:
        _NC_CACHE[key] = build_nc("float32r", with_bias_qkv, with_bias_proj)
    nc = _NC_CACHE[key]

    from concourse.bass_utils import run_bass_kernel_spmd
    res = run_bass_kernel_spmd(nc, in_maps, core_ids=list(range(NCORES)), trace=_trace)

    outs = [res.results[c]["out"] for c in range(NCORES)]
    y = np.empty((B, T, C), dtype=np.float32)
    for b in range(B):
        y[b] = outs[4 * b] + outs[4 * b + 1] + outs[4 * b + 2] + outs[4 * b + 3]
    if _trace:
        return y, res
    return y


# make bass importable at module load so `from kernel import kernel` works
_import_bass()



# revision 2
# speedup vs baseline: 1.0217x; 1.0217x over previous
"""Causal self-attention with sink, sharded over 8 TRN2 NeuronCores.

Sharding: batch x head-group. Core c handles batch b=c//4 and heads
[4*(c%4), 4*(c%4)+4). Each core computes its QKV projection slice,
attention for its 4 heads, and a partial output projection; the host sums
the 4 partials per batch.

Device layout (per core), everything "transposed" (T on the free dim):
  - xT   [C=1024, T=2048]   (host pre-transposed x[b], bf16)
  - qT/kT in SBUF as head-pair tiles [128, T] (2 heads x 64 stacked)
  - v1   [128, 16, 4, 65]   v in natural [t, d] layout per tk-chunk/head
                            plus a ones column (65th) that accumulates the
                            softmax denominator inside the PV matmul
  - S^T = K^T Q per (head, tq-block, tk-chunk) -> exp -> E^T (no max
    subtraction: logits are O(1) for this problem's scale)
  - PV:  out^T[d, tq] (+ denom row) accumulated in PSUM over tk-chunks;
    sink term enters the denominator via a rank-1 matmul
  - normalize via reciprocal + gpsimd partition_broadcast + multiply
  - out projection produces natural [t, co] partials via yT-as-stationary,
    interleaved per tq-block with the attention of the next block
All matmul operands are bfloat16 (1 cycle/row PE streaming); PSUM
accumulation stays fp32. Output DMA'd as bf16; host sums in fp32.
"""

import os
import sys

import numpy as np

B, T, C = 2, 2048, 1024
H, D = 16, 64
NCORES = 8
HLOC = 4           # heads per core
GQ = HLOC * D      # 256 per-core q (or k or v) features
F = 3 * GQ         # 768 per-core qkv features
NCC = C // 128     # 8 contraction chunks
NTQ = T // 512     # 4 query blocks
NTK = T // 128     # 16 key chunks
SCALE = 1.0 / np.sqrt(D)

_BASS_PATHS = ("/opt/trn_rl_repo", "/root/.axon_site/_ro/trn_rl_repo")


def _import_bass():
    for p in _BASS_PATHS:
        if os.path.isdir(p) and p not in sys.path:
            sys.path.insert(0, p)
    import concourse.bass as bass
    import concourse.mybir as mybir
    import concourse.tile as tile
    from concourse import bacc
    return bass, mybir, tile, bacc


def build_nc(mm_dt="bfloat16", with_bias_qkv=True, with_bias_proj=True):
    """Build the per-core Bass program (same program for all 8 cores)."""
    bass, mybir, tile, bacc = _import_bass()
    f32 = mybir.dt.float32
    mdt = getattr(mybir.dt, mm_dt)
    AF = mybir.ActivationFunctionType

    nc = bacc.Bacc("TRN2", target_bir_lowering=False, debug=False)

    xT = nc.dram_tensor("xT", [C, T], mdt, kind="ExternalInput")
    wqkvT = nc.dram_tensor("wqkvT", [C, F], mdt, kind="ExternalInput")
    bqkv = nc.dram_tensor("bqkv", [1, F], mdt, kind="ExternalInput")
    wpT = nc.dram_tensor("wpT", [GQ, C], mdt, kind="ExternalInput")
    bp = nc.dram_tensor("bp", [1, C], mdt, kind="ExternalInput")
    sinkrow = nc.dram_tensor("sinkrow", [1, HLOC * 512], mdt, kind="ExternalInput")
    e65 = nc.dram_tensor("e65", [1, 65], mdt, kind="ExternalInput")
    out = nc.dram_tensor("out", [T, C], mdt, kind="ExternalOutput")

    with tile.TileContext(nc) as tc:
        with (
            tc.tile_pool(name="const", bufs=1) as const,
            tc.tile_pool(name="persist", bufs=1) as persist,
        ):
            # ---- constants ----------------------------------------------
            # (memset does not support bf16, so memset fp32 scratch and
            #  engine-copy into the bf16 tiles)
            ones_f = const.tile([1, 512], f32, tag="ones_f")
            nc.vector.memset(ones_f, 1.0)
            ones_r = const.tile([1, 512], mdt, tag="ones")
            nc.vector.tensor_copy(out=ones_r[:], in_=ones_f[:])
            ones_col = const.tile([128, 1], f32, tag="ones_col")
            nc.vector.memset(ones_col, 1.0)
            e65_r = const.tile([1, 65], mdt, tag="e65")
            nc.sync.dma_start(out=e65_r[:], in_=e65[:, :])
            bqkv_r = const.tile([1, F], mdt, tag="bqkv")
            nc.sync.dma_start(out=bqkv_r[:], in_=bqkv[:, :])
            bp_r = const.tile([1, C], mdt, tag="bp")
            nc.sync.dma_start(out=bp_r[:], in_=bp[:, :])
            sink_r = const.tile([1, HLOC * 512], mdt, tag="sink")
            nc.sync.dma_start(out=sink_r[:], in_=sinkrow[:, :])
            # additive causal mask for the 128x128 diagonal blocks of S^T
            # (tk on partitions, tq on free): keep where tq >= tk.
            tri = const.tile([128, 128], f32, tag="tri")
            nc.gpsimd.memset(tri, 0.0)
            nc.gpsimd.affine_select(
                out=tri, in_=tri,
                compare_op=mybir.AluOpType.is_ge,
                fill=-1e30,
                base=0,
                pattern=[[1, 128]],
                channel_multiplier=-1,
            )

            # ---- persistent activations ---------------------------------
            # qk feature-block tiles: [q01, q23, k01, k23] each [128, T]
            qk = [persist.tile([128, T], mdt, tag=f"qk{i}", name=f"qk{i}") for i in range(4)]
            # v natural + ones column
            v1 = persist.tile([128, NTK, HLOC, 65], mdt, tag="v1", name="v1")
            nc.vector.tensor_copy(
                out=v1[:, :, :, 64:65],
                in_=ones_col[:, :].to_broadcast([128, NTK, HLOC, 1]),
            )
            # normalized attention output, head pairs stacked: yT[hp] [128, T]
            yT = [persist.tile([128, T], mdt, tag=f"yT{i}", name=f"yT{i}") for i in range(2)]

            # ---- input DMA (weights first, then x column-slices) --------
            wq = []
            for i in range(NCC):
                t = persist.tile([128, F], mdt, tag=f"wqkv{i}", name=f"wqkv{i}")
                nc.sync.dma_start(out=t[:], in_=wqkvT[128 * i:128 * (i + 1), :])
                wq.append(t)
            xt = [persist.tile([128, T], mdt, tag=f"xt{i}", name=f"xt{i}")
                  for i in range(NCC)]
            # column-sliced so the first projection block can start after
            # the first 1/4 of x has landed
            for j in range(NTQ):
                for i in range(NCC):
                    nc.sync.dma_start(
                        out=xt[i][:, 512 * j:512 * (j + 1)],
                        in_=xT[128 * i:128 * (i + 1), 512 * j:512 * (j + 1)],
                    )
            wp = []
            for i in range(GQ // 128):
                t = persist.tile([128, C], mdt, tag=f"wp{i}", name=f"wp{i}")
                nc.sync.dma_start(out=t[:], in_=wpT[128 * i:128 * (i + 1), :])
                wp.append(t)

            # ---- q/k projection (transposed out: [feat, t]) -------------
            # feature blocks in wqkvT cols: q:[0,256) k:[256,512) v:[512,768)
            # qk[0]=q01 cols [0,128), qk[1]=q23 [128,256), qk[2]=k01 [256,384), qk[3]=k23 [384,512)
            with tc.tile_pool(name="projps", bufs=4, space="PSUM") as pp:
                fb_order = [0, 2, 1, 3]  # q01, k01, q23, k23
                for fb in fb_order:
                    col0 = [0, 128, 256, 384][fb]
                    dst = [0, 1, 2, 3][fb]
                    for tqi in range(NTQ):
                        ps = pp.tile([128, 512], f32, tag="ps")
                        for cc in range(NCC):
                            nc.tensor.matmul(
                                ps[:, :],
                                wq[cc][:, col0:col0 + 128],
                                xt[cc][:, 512 * tqi:512 * (tqi + 1)],
                                start=(cc == 0),
                                stop=(cc == NCC - 1 and not with_bias_qkv),
                            )
                        if with_bias_qkv:
                            nc.tensor.matmul(
                                ps[:, :],
                                bqkv_r[:, col0:col0 + 128],
                                ones_r[:, 0:512],
                                start=False, stop=True,
                            )
                        nc.any.tensor_copy(
                            out=qk[dst][:, 512 * tqi:512 * (tqi + 1)], in_=ps[:, :]
                        )

                # ---- v projection (natural out: [t, feat]) --------------
                for tb in range(NTK):
                    ps = pp.tile([128, GQ], f32, tag="psv")
                    for cc in range(NCC):
                        nc.tensor.matmul(
                            ps[:, :],
                            xt[cc][:, 128 * tb:128 * (tb + 1)],
                            wq[cc][:, 512:768],
                            start=(cc == 0),
                            stop=(cc == NCC - 1 and not with_bias_qkv),
                        )
                    if with_bias_qkv:
                        nc.tensor.matmul(
                            ps[:, :],
                            ones_r[:, 0:128],
                            bqkv_r[:, 512:768],
                            start=False, stop=True,
                        )
                    nc.any.tensor_copy(
                        out=v1[:, tb, :, 0:64],
                        in_=ps[:].rearrange("p (h d) -> p h d", h=HLOC),
                    )

            # ---- attention + output projection, per tq-block ------------
            with (
                tc.tile_pool(name="spool", bufs=3, space="PSUM") as sp,
                tc.tile_pool(name="pvpool", bufs=3, space="PSUM") as pvp,
                tc.tile_pool(name="oppool", bufs=2, space="PSUM") as op,
                tc.tile_pool(name="epool", bufs=6) as ep,
                tc.tile_pool(name="rpool", bufs=4) as rp,
                tc.tile_pool(name="ostage", bufs=3) as ost,
            ):
                for tqi in range(NTQ):
                    tq0 = 512 * tqi
                    for h in range(HLOC):
                        hp, hs = divmod(h, 2)
                        pb = 64 * hs  # partition base of this head in the pair tiles
                        q_t, k_t = qk[hp], qk[2 + hp]
                        pv = pvp.tile([65, 512], f32, tag="pv")
                        # sink term -> denominator row (also zero-fills rows 0..63)
                        nc.tensor.matmul(
                            pv[:, :], e65_r[:, :], sink_r[0:1, h * 512:(h + 1) * 512],
                            start=True, stop=False,
                        )
                        for tki in range(tq0 // 128 + 4):
                            tk0 = 128 * tki
                            last = tki == tq0 // 128 + 3
                            if tk0 < tq0:
                                # fully-visible block
                                s = sp.tile([128, 512], f32, tag="s")
                                nc.tensor.matmul(
                                    s[:, :],
                                    k_t[pb:pb + 64, tk0:tk0 + 128],
                                    q_t[pb:pb + 64, tq0:tq0 + 512],
                                    start=True, stop=True,
                                )
                                e = ep.tile([128, 512], mdt, tag="e")
                                nc.scalar.activation(out=e[:, :], in_=s[:, :],
                                                     func=AF.Exp, scale=SCALE)
                                nc.tensor.matmul(
                                    pv[:, :], v1[:, tki, h, :], e[:, :],
                                    start=False, stop=last,
                                )
                            else:
                                # diagonal-region block: visible tq cols [tk0, tq0+512)
                                m = (tk0 - tq0) // 128
                                w = 512 - 128 * m
                                s = sp.tile([128, 512], f32, tag="s")
                                nc.tensor.matmul(
                                    s[:, 0:w],
                                    k_t[pb:pb + 64, tk0:tk0 + 128],
                                    q_t[pb:pb + 64, tq0 + 128 * m:tq0 + 512],
                                    start=True, stop=True,
                                )
                                # causal mask on the first 128 visible cols
                                nc.vector.tensor_add(out=s[:, 0:128], in0=s[:, 0:128], in1=tri)
                                e = ep.tile([128, 512], mdt, tag="e")
                                nc.scalar.activation(out=e[:, 0:w], in_=s[:, 0:w],
                                                     func=AF.Exp, scale=SCALE)
                                nc.tensor.matmul(
                                    pv[:, 128 * m:512], v1[:, tki, h, :], e[:, 0:w],
                                    start=False, stop=last,
                                )
                        # normalize: y = out / denom
                        r1 = rp.tile([1, 512], f32, tag="r1")
                        nc.vector.reciprocal(out=r1, in_=pv[64:65, :])
                        rb = rp.tile([64, 512], f32, tag="rb")
                        nc.gpsimd.partition_broadcast(rb, r1)
                        nc.vector.tensor_mul(
                            out=yT[hp][pb:pb + 64, tq0:tq0 + 512],
                            in0=pv[0:64, :],
                            in1=rb,
                        )

                    # ---- output projection for this tq block ------------
                    for tb in range(4 * tqi, 4 * (tqi + 1)):
                        stg = ost.tile([128, C], mdt, tag="ostg")
                        for co in range(2):
                            ps = op.tile([128, 512], f32, tag="ops")
                            for hd in range(2):
                                nc.tensor.matmul(
                                    ps[:, :],
                                    yT[hd][:, 128 * tb:128 * (tb + 1)],
                                    wp[hd][:, 512 * co:512 * (co + 1)],
                                    start=(hd == 0),
                                    stop=(hd == 1 and not with_bias_proj),
                                )
                            if with_bias_proj:
                                nc.tensor.matmul(
                                    ps[:, :],
                                    ones_r[:, 0:128],
                                    bp_r[:, 512 * co:512 * (co + 1)],
                                    start=False, stop=True,
                                )
                            nc.any.tensor_copy(out=stg[:, 512 * co:512 * (co + 1)], in_=ps[:, :])
                        nc.sync.dma_start(out=out[128 * tb:128 * (tb + 1), :], in_=stg[:, :])

    nc.finalize()
    return nc


def make_core_inputs(x, W_qkv, b_qkv, W_proj, b_proj, sink_logit):
    """Host-side sharding: per-core input dicts (host does the transposes)."""
    import ml_dtypes
    bf16 = ml_dtypes.bfloat16

    x = np.asarray(x, dtype=np.float32)
    W_qkv = np.asarray(W_qkv, dtype=np.float32)
    b_qkv = np.asarray(b_qkv, dtype=np.float32)
    W_proj = np.asarray(W_proj, dtype=np.float32)
    b_proj = np.asarray(b_proj, dtype=np.float32)
    sink_logit = np.asarray(sink_logit, dtype=np.float32)

    xTs = [np.ascontiguousarray(x[b].T).astype(bf16) for b in range(B)]
    e65 = np.zeros((1, 65), dtype=np.float32)
    e65[0, 64] = 1.0
    e65 = e65.astype(bf16)

    in_maps = []
    for c in range(NCORES):
        b, g = divmod(c, 4)
        h0 = HLOC * g
        q_rows = slice(GQ * g, GQ * (g + 1))
        k_rows = slice(C + GQ * g, C + GQ * (g + 1))
        v_rows = slice(2 * C + GQ * g, 2 * C + GQ * (g + 1))
        w_slice = np.concatenate(
            [W_qkv[q_rows], W_qkv[k_rows], W_qkv[v_rows]], axis=0
        )  # (768, 1024)
        b_slice = np.concatenate(
            [b_qkv[q_rows], b_qkv[k_rows], b_qkv[v_rows]], axis=0
        )  # (768,)
        sink = np.repeat(
            np.exp(sink_logit[h0:h0 + HLOC]).astype(np.float32)[:, None], 512, axis=1
        ).reshape(1, HLOC * 512)
        in_maps.append({
            "xT": xTs[b],
            "wqkvT": np.ascontiguousarray(w_slice.T).astype(bf16),
            "bqkv": b_slice[None, :].astype(bf16),
            "wpT": np.ascontiguousarray(W_proj[:, q_rows].T).astype(bf16),
            "bp": (b_proj if g == 0 else np.zeros_like(b_proj))[None, :].astype(bf16),
            "sinkrow": sink.astype(bf16),
            "e65": e65,
        })
    return in_maps


_NC_CACHE = {}


def kernel(x, W_qkv, b_qkv, W_proj, b_proj, sink_logit, _trace=False):
    from concourse.bass_utils import run_bass_kernel_spmd  # noqa: F401 (path set below)

    in_maps = make_core_inputs(x, W_qkv, b_qkv, W_proj, b_proj, sink_logit)
    with_bias_qkv = bool(np.any(np.asarray(b_qkv)))
    with_bias_proj = bool(np.any(np.asarray(b_proj)))
    key = ("bfloat16", with_bias_qkv, with_bias_proj)
    if key not in _NC_CACHE:
        _NC_CACHE[key] = build_nc("bfloat16", with_bias_qkv, with_bias_proj)
    nc = _NC_CACHE[key]

    from concourse.bass_utils import run_bass_kernel_spmd
    res = run_bass_kernel_spmd(nc, in_maps, core_ids=list(range(NCORES)), trace=_trace)

    outs = [np.asarray(res.results[c]["out"], dtype=np.float32) for c in range(NCORES)]
    y = np.empty((B, T, C), dtype=np.float32)
    for b in range(B):
        y[b] = outs[4 * b] + outs[4 * b + 1] + outs[4 * b + 2] + outs[4 * b + 3]
    if _trace:
        return y, res
    return y


# make bass importable at module load so `from kernel import kernel` works
_import_bass()


# revision 4
# speedup vs baseline: 1.6225x; 1.5880x over previous
"""Causal self-attention with sink, sharded over 8 TRN2 NeuronCores.

Sharding: batch x head-group. Core c handles batch b=c//4 and heads
[4*(c%4), 4*(c%4)+4). Each core computes its QKV projection slice,
attention for its 4 heads, and a partial output projection; the host sums
the 4 partials per batch.

Device layout (per core), everything "transposed" (T on the free dim):
  - xT   [C=1024, T=2048]   (host pre-transposed x[b], bf16)
  - qT/kT in SBUF as head-pair tiles [128, T] (2 heads x 64 stacked)
  - v1   [128, 16, 4, 65]   v in natural [t, d] layout per tk-chunk/head
                            plus a ones column (65th) that accumulates the
                            softmax denominator inside the PV matmul
  - S^T = K^T Q per (head, tq-block, tk-chunk) -> exp -> E^T (no max
    subtraction: logits are O(1) for this problem's scale)
  - PV:  out^T[d, tq] (+ denom row) accumulated in PSUM over tk-chunks
  - exp(sink) joins the denominator via a vector add just before the
    (fast approx) reciprocal; gpsimd broadcast + multiply normalizes
  - out projection produces natural [t, co] partials via yT-as-stationary,
    software-pipelined one tq-block behind the attention loop
All matmul operands are bfloat16; PSUM accumulation stays fp32. Output is
DMA'd as bf16; the host sums partials in fp32.
"""

import os
import sys

import numpy as np

B, T, C = 2, 2048, 1024
H, D = 16, 64
NCORES = 8
HLOC = 4           # heads per core
GQ = HLOC * D      # 256 per-core q (or k or v) features
F = 3 * GQ         # 768 per-core qkv features
NCC = C // 128     # 8 contraction chunks
NTQ = T // 512     # 4 query blocks
NTK = T // 128     # 16 key chunks
SCALE = 1.0 / np.sqrt(D)

_BASS_PATHS = ("/opt/trn_rl_repo", "/root/.axon_site/_ro/trn_rl_repo")


def _import_bass():
    for p in _BASS_PATHS:
        if os.path.isdir(p) and p not in sys.path:
            sys.path.insert(0, p)
    import concourse.bass as bass
    import concourse.mybir as mybir
    import concourse.tile as tile
    from concourse import bacc
    return bass, mybir, tile, bacc


def build_nc(mm_dt="bfloat16", with_bias_qkv=True, with_bias_proj=True):
    """Build the per-core Bass program (same program for all 8 cores)."""
    bass, mybir, tile, bacc = _import_bass()
    f32 = mybir.dt.float32
    mdt = getattr(mybir.dt, mm_dt)
    AF = mybir.ActivationFunctionType

    nc = bacc.Bacc("TRN2", target_bir_lowering=False, debug=False)

    xT = nc.dram_tensor("xT", [C, T], mdt, kind="ExternalInput")
    wqkvT = nc.dram_tensor("wqkvT", [C, F], mdt, kind="ExternalInput")
    bqkv = nc.dram_tensor("bqkv", [1, F], mdt, kind="ExternalInput")
    wpT = nc.dram_tensor("wpT", [GQ, C], mdt, kind="ExternalInput")
    bp = nc.dram_tensor("bp", [1, C], mdt, kind="ExternalInput")
    sink4 = nc.dram_tensor("sink4", [1, HLOC], f32, kind="ExternalInput")
    out = nc.dram_tensor("out", [T, C], mdt, kind="ExternalOutput")

    with tile.TileContext(nc) as tc:
        with (
            tc.tile_pool(name="const", bufs=1) as const,
            tc.tile_pool(name="persist", bufs=1) as persist,
        ):
            # ---- constants ----------------------------------------------
            ones_f = const.tile([1, 512], f32, tag="ones_f")
            nc.vector.memset(ones_f, 1.0)
            ones_r = const.tile([1, 512], mdt, tag="ones")
            nc.vector.tensor_copy(out=ones_r[:], in_=ones_f[:])
            ones_col = const.tile([128, 1], f32, tag="ones_col")
            nc.vector.memset(ones_col, 1.0)
            bqkv_r = const.tile([1, F], mdt, tag="bqkv")
            nc.sync.dma_start(out=bqkv_r[:], in_=bqkv[:, :])
            bp_r = const.tile([1, C], mdt, tag="bp")
            nc.sync.dma_start(out=bp_r[:], in_=bp[:, :])
            sink_r = const.tile([1, HLOC], f32, tag="sink")
            nc.sync.dma_start(out=sink_r[:], in_=sink4[:, :])
            # additive causal mask for the 128x128 diagonal blocks of S^T
            # (tk on partitions, tq on free): keep where tq >= tk.
            tri = const.tile([128, 128], f32, tag="tri")
            nc.gpsimd.memset(tri, 0.0)
            nc.gpsimd.affine_select(
                out=tri, in_=tri,
                compare_op=mybir.AluOpType.is_ge,
                fill=-1e30,
                base=0,
                pattern=[[1, 128]],
                channel_multiplier=-1,
            )

            # ---- persistent activations ---------------------------------
            # qk feature-block tiles: [q01, q23, k01, k23] each [128, T]
            qk = [persist.tile([128, T], mdt, tag=f"qk{i}", name=f"qk{i}") for i in range(4)]
            # v natural + ones column
            v1 = persist.tile([128, NTK, HLOC, 65], mdt, tag="v1", name="v1")
            nc.vector.tensor_copy(
                out=v1[:, :, :, 64:65],
                in_=ones_col[:, :].to_broadcast([128, NTK, HLOC, 1]),
            )
            # normalized attention output, head pairs stacked: yT[hp] [128, T]
            yT = [persist.tile([128, T], mdt, tag=f"yT{i}", name=f"yT{i}") for i in range(2)]

            # ---- input DMA (weights first, then x column-slices) --------
            wq = []
            for i in range(NCC):
                t = persist.tile([128, F], mdt, tag=f"wqkv{i}", name=f"wqkv{i}")
                nc.sync.dma_start(out=t[:], in_=wqkvT[128 * i:128 * (i + 1), :])
                wq.append(t)
            xt = [persist.tile([128, T], mdt, tag=f"xt{i}", name=f"xt{i}")
                  for i in range(NCC)]
            # column-sliced so the first projection block can start after
            # the first 1/4 of x has landed
            for j in range(NTQ):
                for i in range(NCC):
                    nc.sync.dma_start(
                        out=xt[i][:, 512 * j:512 * (j + 1)],
                        in_=xT[128 * i:128 * (i + 1), 512 * j:512 * (j + 1)],
                    )
            wp = []
            for i in range(GQ // 128):
                t = persist.tile([128, C], mdt, tag=f"wp{i}", name=f"wp{i}")
                nc.sync.dma_start(out=t[:], in_=wpT[128 * i:128 * (i + 1), :])
                wp.append(t)

            # ---- q/k projection (transposed out: [feat, t]) -------------
            # feature blocks in wqkvT cols: q:[0,256) k:[256,512) v:[512,768)
            # qk[0]=q01 cols [0,128), qk[1]=q23 [128,256), qk[2]=k01 [256,384), qk[3]=k23 [384,512)
            with tc.tile_pool(name="projps", bufs=4, space="PSUM") as pp:
                fb_order = [0, 2, 1, 3]  # q01, k01, q23, k23
                for fb in fb_order:
                    col0 = [0, 128, 256, 384][fb]
                    dst = [0, 1, 2, 3][fb]
                    for tqi in range(NTQ):
                        ps = pp.tile([128, 512], f32, tag="ps")
                        for cc in range(NCC):
                            nc.tensor.matmul(
                                ps[:, :],
                                wq[cc][:, col0:col0 + 128],
                                xt[cc][:, 512 * tqi:512 * (tqi + 1)],
                                start=(cc == 0),
                                stop=(cc == NCC - 1 and not with_bias_qkv),
                            )
                        if with_bias_qkv:
                            nc.tensor.matmul(
                                ps[:, :],
                                bqkv_r[:, col0:col0 + 128],
                                ones_r[:, 0:512],
                                start=False, stop=True,
                            )
                        nc.vector.tensor_copy(
                            out=qk[dst][:, 512 * tqi:512 * (tqi + 1)], in_=ps[:, :]
                        )

                # ---- v projection (natural out: [t, feat]) --------------
                for tb in range(NTK):
                    ps = pp.tile([128, GQ], f32, tag="psv")
                    for cc in range(NCC):
                        nc.tensor.matmul(
                            ps[:, :],
                            xt[cc][:, 128 * tb:128 * (tb + 1)],
                            wq[cc][:, 512:768],
                            start=(cc == 0),
                            stop=(cc == NCC - 1 and not with_bias_qkv),
                        )
                    if with_bias_qkv:
                        nc.tensor.matmul(
                            ps[:, :],
                            ones_r[:, 0:128],
                            bqkv_r[:, 512:768],
                            start=False, stop=True,
                        )
                    nc.vector.tensor_copy(
                        out=v1[:, tb, :, 0:64],
                        in_=ps[:].rearrange("p (h d) -> p h d", h=HLOC),
                    )

            # ---- attention + output projection, software-pipelined ------
            with (
                tc.tile_pool(name="spool", bufs=3, space="PSUM") as sp,
                tc.tile_pool(name="pvpool", bufs=3, space="PSUM") as pvp,
                tc.tile_pool(name="oppool", bufs=2, space="PSUM") as op,
                tc.tile_pool(name="epool", bufs=6) as ep,
                tc.tile_pool(name="rpool", bufs=4) as rp,
                tc.tile_pool(name="ostage", bufs=3) as ost,
            ):
                def attention_block(tqi):
                    tq0 = 512 * tqi
                    for h in range(HLOC):
                        hp, hs = divmod(h, 2)
                        pb = 64 * hs  # partition base of this head in the pair tiles
                        q_t, k_t = qk[hp], qk[2 + hp]
                        pv = pvp.tile([65, 512], f32, tag="pv")
                        for tki in range(tq0 // 128 + 4):
                            tk0 = 128 * tki
                            first = tki == 0
                            last = tki == tq0 // 128 + 3
                            if tk0 < tq0:
                                # fully-visible block
                                s = sp.tile([128, 512], f32, tag="s")
                                nc.tensor.matmul(
                                    s[:, :],
                                    k_t[pb:pb + 64, tk0:tk0 + 128],
                                    q_t[pb:pb + 64, tq0:tq0 + 512],
                                    start=True, stop=True,
                                )
                                e = ep.tile([128, 512], mdt, tag="e")
                                nc.scalar.activation(out=e[:, :], in_=s[:, :],
                                                     func=AF.Exp, scale=SCALE)
                                nc.tensor.matmul(
                                    pv[:, :], v1[:, tki, h, :], e[:, :],
                                    start=first, stop=last,
                                )
                            else:
                                # diagonal-region block: visible tq cols [tk0, tq0+512)
                                m = (tk0 - tq0) // 128
                                w = 512 - 128 * m
                                s = sp.tile([128, 512], f32, tag="s")
                                nc.tensor.matmul(
                                    s[:, 0:w],
                                    k_t[pb:pb + 64, tk0:tk0 + 128],
                                    q_t[pb:pb + 64, tq0 + 128 * m:tq0 + 512],
                                    start=True, stop=True,
                                )
                                # causal mask on the first 128 visible cols
                                nc.vector.tensor_add(out=s[:, 0:128], in0=s[:, 0:128], in1=tri)
                                e = ep.tile([128, 512], mdt, tag="e")
                                nc.scalar.activation(out=e[:, 0:w], in_=s[:, 0:w],
                                                     func=AF.Exp, scale=SCALE)
                                nc.tensor.matmul(
                                    pv[:, 128 * m:512], v1[:, tki, h, :], e[:, 0:w],
                                    start=first, stop=last,
                                )
                        # normalize: y = out / (denom + exp(sink))
                        r1a = rp.tile([1, 512], f32, tag="r1a")
                        nc.vector.tensor_scalar_add(
                            out=r1a, in0=pv[64:65, :], scalar1=sink_r[0:1, h:h + 1]
                        )
                        r1 = rp.tile([1, 512], f32, tag="r1")
                        nc.vector.reciprocal_approx_fast(out=r1, in_=r1a)
                        rb = rp.tile([64, 512], f32, tag="rb")
                        nc.gpsimd.partition_broadcast(rb, r1)
                        nc.vector.tensor_mul(
                            out=yT[hp][pb:pb + 64, tq0:tq0 + 512],
                            in0=pv[0:64, :],
                            in1=rb,
                        )

                def outproj_block(tqi):
                    for tb in range(4 * tqi, 4 * (tqi + 1)):
                        stg = ost.tile([128, C], mdt, tag="ostg")
                        for co in range(2):
                            ps = op.tile([128, 512], f32, tag="ops")
                            for hd in range(2):
                                nc.tensor.matmul(
                                    ps[:, :],
                                    yT[hd][:, 128 * tb:128 * (tb + 1)],
                                    wp[hd][:, 512 * co:512 * (co + 1)],
                                    start=(hd == 0),
                                    stop=(hd == 1 and not with_bias_proj),
                                )
                            if with_bias_proj:
                                nc.tensor.matmul(
                                    ps[:, :],
                                    ones_r[:, 0:128],
                                    bp_r[:, 512 * co:512 * (co + 1)],
                                    start=False, stop=True,
                                )
                            nc.vector.tensor_copy(out=stg[:, 512 * co:512 * (co + 1)], in_=ps[:, :])
                        nc.sync.dma_start(out=out[128 * tb:128 * (tb + 1), :], in_=stg[:, :])

                # pipeline: outproj trails attention by one tq block, so the
                # normalize chain of block i overlaps attention of block i+1
                attention_block(0)
                for tqi in range(1, NTQ):
                    attention_block(tqi)
                    outproj_block(tqi - 1)
                outproj_block(NTQ - 1)

    nc.finalize()
    return nc


def make_core_inputs(x, W_qkv, b_qkv, W_proj, b_proj, sink_logit):
    """Host-side sharding: per-core input dicts (host does the transposes)."""
    import ml_dtypes
    bf16 = ml_dtypes.bfloat16

    x = np.asarray(x, dtype=np.float32)
    W_qkv = np.asarray(W_qkv, dtype=np.float32)
    b_qkv = np.asarray(b_qkv, dtype=np.float32)
    W_proj = np.asarray(W_proj, dtype=np.float32)
    b_proj = np.asarray(b_proj, dtype=np.float32)
    sink_logit = np.asarray(sink_logit, dtype=np.float32)

    xTs = [np.ascontiguousarray(x[b].T).astype(bf16) for b in range(B)]

    in_maps = []
    for c in range(NCORES):
        b, g = divmod(c, 4)
        h0 = HLOC * g
        q_rows = slice(GQ * g, GQ * (g + 1))
        k_rows = slice(C + GQ * g, C + GQ * (g + 1))
        v_rows = slice(2 * C + GQ * g, 2 * C + GQ * (g + 1))
        w_slice = np.concatenate(
            [W_qkv[q_rows], W_qkv[k_rows], W_qkv[v_rows]], axis=0
        )  # (768, 1024)
        b_slice = np.concatenate(
            [b_qkv[q_rows], b_qkv[k_rows], b_qkv[v_rows]], axis=0
        )  # (768,)
        sink = np.exp(sink_logit[h0:h0 + HLOC]).astype(np.float32)[None, :]
        in_maps.append({
            "xT": xTs[b],
            "wqkvT": np.ascontiguousarray(w_slice.T).astype(bf16),
            "bqkv": b_slice[None, :].astype(bf16),
            "wpT": np.ascontiguousarray(W_proj[:, q_rows].T).astype(bf16),
            "bp": (b_proj if g == 0 else np.zeros_like(b_proj))[None, :].astype(bf16),
            "sink4": sink,
        })
    return in_maps


_NC_CACHE = {}


def kernel(x, W_qkv, b_qkv, W_proj, b_proj, sink_logit, _trace=False):
    from concourse.bass_utils import run_bass_kernel_spmd  # noqa: F401 (path set below)

    in_maps = make_core_inputs(x, W_qkv, b_qkv, W_proj, b_proj, sink_logit)
    with_bias_qkv = bool(np.any(np.asarray(b_qkv)))
    with_bias_proj = bool(np.any(np.asarray(b_proj)))
    key = ("bfloat16", with_bias_qkv, with_bias_proj)
    if key not in _NC_CACHE:
        _NC_CACHE[key] = build_nc("bfloat16", with_bias_qkv, with_bias_proj)
    nc = _NC_CACHE[key]

    from concourse.bass_utils import run_bass_kernel_spmd
    res = run_bass_kernel_spmd(nc, in_maps, core_ids=list(range(NCORES)), trace=_trace)

    outs = [np.asarray(res.results[c]["out"], dtype=np.float32) for c in range(NCORES)]
    y = np.empty((B, T, C), dtype=np.float32)
    for b in range(B):
        y[b] = outs[4 * b] + outs[4 * b + 1] + outs[4 * b + 2] + outs[4 * b + 3]
    if _trace:
        return y, res
    return y


# make bass importable at module load so `from kernel import kernel` works
_import_bass()
